# revision 1
# baseline (speedup 1.0000x reference)
"""Trainium2 Bass kernel for the NeuralJumpODE problem.

Math
----
reference() scans over observations, but the carried ODE state h is
OVERWRITTEN by jump_nn(x_i) at every observation: each (batch, obs)
pair is independent. The scan carry only shifts outputs by one step:
    preds[b, i]        = output_nn(jump_nn(x[b, i]))
    preds_before[b, 0] = 0
    preds_before[b, i] = output_nn(H(b, i-1))          for i >= 1
where H(b, i) = jump state integrated through n_steps Euler substeps.

So the whole problem flattens to 512*64 = 32768 independent rows with
10 sequential Euler substeps each. Data-parallel over 8 cores:
4096 rows/core.

Device layout (per core)
------------------------
Everything is stored transposed: features on SBUF partitions, rows on
the free dimension. A linear layer Z = A @ W becomes
Z^T = W^T A^T = matmul(lhsT=W_chunk[K,128], rhs=A^T_chunk[K,512]).
All matmul operands are float32r (full-rate PE, ~1.5e-4 rel error).

Bias / time folding (all exact, host-side, float64):
  * every tanh-layer bias is applied via the ScalarE activation bias
    (per-partition = per-feature in this layout),
  * ode pre-activation needs fW1_t * (t_i + s*dt) + fW1_dt * dt;
    t_i and dt are static moving rows, the s-dependence is folded into
    per-substep weight variants of the small 34-row "chunk 4",
  * fb2 (the non-tanh ode bias) is handled by deficiency tracking:
    we integrate G_s = H_s - s*(fb2 x dt); its effect on the next
    pre-activation is s*(fW1_h^T fb2) x dt, folded into the dt-row
    weights; the final output layer gets a K=1 correction matmul with
    weight S*(oW1^T fb2) against the dt row,
  * ob2 is added on the host after gathering.
"""

import numpy as np

import concourse.bacc as bacc
import concourse.mybir as mybir
import concourse.tile as tile
from concourse.bass_utils import run_bass_kernel_spmd

F32 = mybir.dt.float32
F32R = mybir.dt.float32r
TANH = mybir.ActivationFunctionType.Tanh

B, NOBS, DX, HID, DY = 512, 64, 32, 512, 32
NCORES = 8
BLOC = B // NCORES            # trajectories per core
R = BLOC * NOBS               # rows per core (4096)
RT = 512                      # rows per row-tile
NRT = R // RT                 # row-tiles per core (8)
NCH = HID // 128              # 128-feature chunks (4)
M4P = DX + 2                  # moving chunk-4 partitions: x, t_i, dt

_prog_cache = {}


def _build(S, loop_n=None):
    nc = bacc.Bacc("TRN2", target_bir_lowering=False, debug=False)

    m4d = nc.dram_tensor("m4", [M4P, R], F32R, kind="ExternalInput")
    dtbd = nc.dram_tensor("dtb", [128, R], F32R, kind="ExternalInput")
    wj1d = nc.dram_tensor("wj1", [DX, HID], F32R, kind="ExternalInput")
    wj2d = nc.dram_tensor("wj2", [128, NCH * HID], F32R, kind="ExternalInput")
    wfhd = nc.dram_tensor("wfh", [128, NCH * HID], F32R, kind="ExternalInput")
    wf4d = nc.dram_tensor("wf4", [M4P, S * HID], F32R, kind="ExternalInput")
    wf2d = nc.dram_tensor("wf2", [128, NCH * HID], F32R, kind="ExternalInput")
    wo1d = nc.dram_tensor("wo1", [128, NCH * HID], F32R, kind="ExternalInput")
    wodtd = nc.dram_tensor("wodt", [1, HID], F32R, kind="ExternalInput")
    wo2d = nc.dram_tensor("wo2", [128, NCH * DY], F32R, kind="ExternalInput")
    biasd = nc.dram_tensor("bias", [128, 16], F32, kind="ExternalInput")
    yjd = nc.dram_tensor("yj", [DY, R], F32, kind="ExternalOutput")
    ybd = nc.dram_tensor("yb", [DY, R], F32, kind="ExternalOutput")

    with tile.TileContext(nc) as tc:
        with (
            tc.tile_pool(name="const", bufs=1) as cp,
            tc.tile_pool(name="work", bufs=3) as wp,
            tc.tile_pool(name="psum", bufs=4, space="PSUM") as pp,
        ):
            wj1 = cp.tile([DX, HID], F32R, name="wj1s")
            wj2 = cp.tile([128, NCH * HID], F32R, name="wj2s")
            wfh = cp.tile([128, NCH * HID], F32R, name="wfhs")
            wf4 = cp.tile([M4P, S * HID], F32R, name="wf4s")
            wf2 = cp.tile([128, NCH * HID], F32R, name="wf2s")
            wo1 = cp.tile([128, NCH * HID], F32R, name="wo1s")
            wodt = cp.tile([1, HID], F32R, name="wodts")
            wo2 = cp.tile([128, NCH * DY], F32R, name="wo2s")
            bias = cp.tile([128, 16], F32, name="biass")
            for sb, dr in ((wj1, wj1d), (wj2, wj2d), (wfh, wfhd),
                           (wf4, wf4d), (wf2, wf2d), (wo1, wo1d),
                           (wodt, wodtd), (wo2, wo2d), (bias, biasd)):
                nc.sync.dma_start(sb[:], dr[:])

            JB1, JB2, FB1, OB1 = 0, 4, 8, 12  # bias column groups

            def body():
                for pair in range(NRT // 2):
                    rts = (2 * pair, 2 * pair + 1)
                    m4 = {}
                    dtb = {}
                    g = {}
                    t = {}
                    td = {}
                    for rt in rts:
                        m4[rt] = wp.tile([M4P, RT], F32R, name="m4t",
                                         tag="m4t")
                        dtb[rt] = wp.tile([128, RT], F32R, name="dtbt",
                                          tag="dtbt")
                        g[rt] = wp.tile([128, NCH * RT], F32R, name="gt",
                                        tag="gt")
                        t[rt] = wp.tile([128, NCH * RT], F32R, name="tt",
                                        tag="tt")
                        td[rt] = wp.tile([128, NCH * RT], F32R, name="tdt",
                                         tag="tdt")
                        nc.sync.dma_start(m4[rt][:],
                                          m4d[:, rt * RT:(rt + 1) * RT])
                        nc.sync.dma_start(dtb[rt][:],
                                          dtbd[:, rt * RT:(rt + 1) * RT])

                    def lay(dst, w, rhs_chunks, bias_col, extra=None):
                        """tanh layer: dst chunks = tanh(W^T @ rhs + bias).

                        w: (128, NCH*HID) chunked weights; rhs_chunks:
                        list of NCH (128, RT) APs; extra: (lhsT, rhs)
                        appended to the contraction. Returns psum tiles.
                        """
                        for c in range(NCH):
                            ps = pp.tile([128, RT], F32, name="p1",
                                         tag="p1")
                            nk = len(rhs_chunks)
                            last = nk - 1 if extra is None else nk
                            for k in range(nk):
                                nc.tensor.matmul(
                                    ps[:],
                                    w[:, k * HID + c * 128:
                                      k * HID + (c + 1) * 128],
                                    rhs_chunks[k],
                                    start=(k == 0), stop=(k == last))
                            if extra is not None:
                                elhs, erhs = extra
                                nc.tensor.matmul(
                                    ps[:],
                                    elhs[:, c * 128:(c + 1) * 128],
                                    erhs,
                                    start=False, stop=True)
                            nc.scalar.activation(
                                dst[:, c * RT:(c + 1) * RT], ps[:], TANH,
                                bias=bias[:, bias_col + c:bias_col + c + 1])

                    def out_layer(rt, dst_dram, aug):
                        """y^T = tanh(oW1^T G + ob1) -> oW2; DMA out."""
                        gch = [g[rt][:, k * RT:(k + 1) * RT]
                               for k in range(NCH)]
                        extra = None
                        if aug:
                            extra = (wodt, dtb[rt][0:1, :])
                        lay(t[rt], wo1, gch, OB1, extra=extra)
                        ps = pp.tile([DY, RT], F32, name="p2", tag="p2")
                        for k in range(NCH):
                            nc.tensor.matmul(
                                ps[:],
                                wo2[:, k * DY:(k + 1) * DY],
                                t[rt][:, k * RT:(k + 1) * RT],
                                start=(k == 0), stop=(k == NCH - 1))
                        yt = wp.tile([DY, RT], F32, name="yt", tag="yt")
                        nc.scalar.copy(yt[:], ps[:])
                        nc.sync.dma_start(
                            dst_dram[:, rt * RT:(rt + 1) * RT], yt[:])

                    # jump network + first prediction
                    for rt in rts:
                        # JL1: K=DX single chunk
                        for c in range(NCH):
                            ps = pp.tile([128, RT], F32, name="p1", tag="p1")
                            nc.tensor.matmul(
                                ps[:], wj1[:, c * 128:(c + 1) * 128],
                                m4[rt][0:DX, :], start=True, stop=True)
                            nc.scalar.activation(
                                t[rt][:, c * RT:(c + 1) * RT], ps[:], TANH,
                                bias=bias[:, JB1 + c:JB1 + c + 1])
                        # JL2 -> G = H0
                        tch = [t[rt][:, k * RT:(k + 1) * RT]
                               for k in range(NCH)]
                        for c in range(NCH):
                            ps = pp.tile([128, RT], F32, name="p2", tag="p2")
                            for k in range(NCH):
                                nc.tensor.matmul(
                                    ps[:],
                                    wj2[:, k * HID + c * 128:
                                        k * HID + (c + 1) * 128],
                                    tch[k], start=(k == 0),
                                    stop=(k == NCH - 1))
                            nc.scalar.activation(
                                g[rt][:, c * RT:(c + 1) * RT], ps[:], TANH,
                                bias=bias[:, JB2 + c:JB2 + c + 1])
                    for rt in rts:
                        out_layer(rt, yjd, aug=False)

                    # Euler substeps
                    for s in range(S):
                        for rt in rts:
                            gch = [g[rt][:, k * RT:(k + 1) * RT]
                                   for k in range(NCH)]
                            lay(t[rt], wfh, gch, FB1,
                                extra=(wf4[:, s * HID:(s + 1) * HID],
                                       m4[rt][:]))
                            for c in range(NCH):
                                nc.vector.tensor_mul(
                                    td[rt][:, c * RT:(c + 1) * RT],
                                    t[rt][:, c * RT:(c + 1) * RT],
                                    dtb[rt][:])
                            tdch = [td[rt][:, k * RT:(k + 1) * RT]
                                    for k in range(NCH)]
                            for c in range(NCH):
                                ps = pp.tile([128, RT], F32, name="p2",
                                             tag="p2")
                                for k in range(NCH):
                                    nc.tensor.matmul(
                                        ps[:],
                                        wf2[:, k * HID + c * 128:
                                            k * HID + (c + 1) * 128],
                                        tdch[k], start=(k == 0),
                                        stop=(k == NCH - 1))
                                nc.vector.tensor_add(
                                    g[rt][:, c * RT:(c + 1) * RT],
                                    g[rt][:, c * RT:(c + 1) * RT],
                                    ps[:])

                    # prediction just before the next jump
                    for rt in rts:
                        out_layer(rt, ybd, aug=True)

            if loop_n is None:
                body()
            else:
                with tc.For_i(0, loop_n, 1):
                    body()

    nc.compile()
    return nc


def _chunk(w):
    """(K, N) -> (128, (K/128)*N), K-chunk kc at columns [kc*N:(kc+1)*N]."""
    k, n = w.shape
    return np.ascontiguousarray(
        w.reshape(k // 128, 128, n).transpose(1, 0, 2).reshape(128, -1))


def _prepare(times, values, jW1, jb1, jW2, jb2, fW1, fb1, fW2, fb2,
             oW1, ob1, oW2, ob2, S):
    f32 = np.float32
    t_next = np.concatenate([times[:, 1:], times[:, -1:]], axis=1)
    dt = ((t_next - times) / f32(S)).astype(f32)

    w64 = {k: np.asarray(v, np.float64) for k, v in dict(
        jW1=jW1, jW2=jW2, fW1=fW1, fW2=fW2, oW1=oW1, oW2=oW2,
        jb1=jb1, jb2=jb2, fb1=fb1, fb2=fb2, ob1=ob1).items()}
    v = w64["fW1"][:HID].T @ w64["fb2"]          # (HID,)

    wf4 = np.empty((M4P, S * HID), f32)
    for s in range(S):
        blk = np.empty((M4P, HID), np.float64)
        blk[0:DX] = w64["fW1"][HID:HID + DX]
        blk[DX] = w64["fW1"][HID + DX]                       # t_i row
        blk[DX + 1] = (w64["fW1"][HID + DX + 1]
                       + s * (w64["fW1"][HID + DX] + v))     # dt row
        wf4[:, s * HID:(s + 1) * HID] = blk.astype(f32)

    wodt = (f32(S) * (w64["oW1"].T @ w64["fb2"])).astype(f32)[None, :]

    bias = np.zeros((128, 16), f32)
    for col, b in ((0, jb1), (4, jb2), (8, fb1), (12, ob1)):
        bias[:, col:col + 4] = np.asarray(b, f32).reshape(NCH, 128).T

    shared = {
        "wj1": np.ascontiguousarray(np.asarray(jW1, f32)),
        "wj2": _chunk(np.asarray(jW2, f32)),
        "wfh": _chunk(np.asarray(fW1, f32)[:HID]),
        "wf4": wf4,
        "wf2": _chunk(np.asarray(fW2, f32)),
        "wo1": _chunk(np.asarray(oW1, f32)),
        "wodt": wodt,
        "wo2": _chunk(np.asarray(oW2, f32)),
        "bias": bias,
    }

    in_maps = []
    for c in range(NCORES):
        sl = slice(c * BLOC, (c + 1) * BLOC)
        m4 = np.empty((M4P, R), f32)
        m4[0:DX] = values[sl].reshape(R, DX).T
        m4[DX] = times[sl].reshape(R)
        m4[DX + 1] = dt[sl].reshape(R)
        dtb = np.ascontiguousarray(
            np.broadcast_to(dt[sl].reshape(R), (128, R)))
        in_maps.append({"m4": m4, "dtb": dtb, **shared})
    return in_maps


def _assemble(results, ob2):
    f32 = np.float32
    ob2 = np.asarray(ob2, f32)

    def gather(name):
        arr = np.stack([results[c][name] for c in range(NCORES)])
        return (arr.transpose(0, 2, 1)              # (8, R, DY)
                .reshape(B, NOBS, DY).astype(f32))

    preds = gather("yj") + ob2
    yb = gather("yb") + ob2
    pb = np.zeros_like(preds)
    pb[:, 1:] = yb[:, :-1]
    return preds, pb


def run_on_hw(inputs, loop_n=None, **run_kwargs):
    """Returns (in_maps, BassKernelResults). loop_n repeats the body
    on-device (for timing)."""
    times = np.asarray(inputs["times"], np.float32)
    values = np.asarray(inputs["values"], np.float32)
    S = int(inputs["n_steps"])
    key = (S, loop_n)
    if key not in _prog_cache:
        _prog_cache[key] = _build(S, loop_n=loop_n)
    nc = _prog_cache[key]
    in_maps = _prepare(
        times, values, inputs["jW1"], inputs["jb1"], inputs["jW2"],
        inputs["jb2"], inputs["fW1"], inputs["fb1"], inputs["fW2"],
        inputs["fb2"], inputs["oW1"], inputs["ob1"], inputs["oW2"],
        inputs["ob2"], S)
    res = run_bass_kernel_spmd(nc, in_maps, core_ids=list(range(NCORES)),
                               **run_kwargs)
    return res


def kernel(**inputs):
    res = run_on_hw(inputs)
    return _assemble(res.results, inputs["ob2"])


# revision 7
# speedup vs baseline: 4.4164x; 4.4164x over previous
"""Trainium2 Bass kernel for the NeuralJumpODE problem.

Math
----
reference() scans over observations, but the carried ODE state h is
OVERWRITTEN by jump_nn(x_i) at every observation: each (batch, obs)
pair is independent. The scan carry only shifts outputs by one step:
    preds[b, i]        = output_nn(jump_nn(x[b, i]))
    preds_before[b, 0] = 0
    preds_before[b, i] = output_nn(H(b, i-1))          for i >= 1
where H(b, i) = jump state integrated through n_steps Euler substeps.

So the whole problem flattens to 512*64 = 32768 independent rows with
10 sequential Euler substeps each. Data-parallel over 8 cores:
4096 rows/core.

Device layout (per core)
------------------------
Everything is stored transposed: features on SBUF partitions, rows on
the free dimension. A linear layer Z = A @ W becomes
Z^T = W^T A^T = matmul(lhsT=W_chunk[K,128], rhs=A^T_chunk[K,512]).
All matmul operands are float32r (full-rate PE, ~1.5e-4 rel error).

Bias / time folding (all exact, host-side, float64):
  * every tanh-layer bias is applied via the ScalarE activation bias
    (per-partition = per-feature in this layout),
  * ode pre-activation needs fW1_t * (t_i + s*dt) + fW1_dt * dt;
    t_i and dt are static moving rows, the s-dependence is folded into
    per-substep weight variants of the small 34-row "chunk 4",
  * fb2 (the non-tanh ode bias) is handled by deficiency tracking:
    we integrate G_s = H_s - s*(fb2 x dt); its effect on the next
    pre-activation is s*(fW1_h^T fb2) x dt, folded into the dt-row
    weights; the final output layer gets a K=1 correction matmul with
    weight S*(oW1^T fb2) against the dt row,
  * ob2 is added on the host after gathering.
"""

import numpy as np

import concourse.bacc as bacc
import concourse.mybir as mybir
import concourse.tile as tile
from concourse.bass_utils import run_bass_kernel_spmd

F32 = mybir.dt.float32
F32R = mybir.dt.float32r
TANH = mybir.ActivationFunctionType.Tanh

B, NOBS, DX, HID, DY = 512, 64, 32, 512, 32
NCORES = 8
BLOC = B // NCORES            # trajectories per core
R = BLOC * NOBS               # rows per core (4096)
RT = 512                      # rows per row-tile
NRT = R // RT                 # row-tiles per core (8)
NCH = HID // 128              # 128-feature chunks (4)
M4P = DX + 2                  # moving chunk-4 partitions: x, t_i, dt

_prog_cache = {}


def _build(S, loop_n=None, use_aug=True):
    nc = bacc.Bacc("TRN2", target_bir_lowering=False, debug=False)

    m4d = nc.dram_tensor("m4", [M4P, R], F32R, kind="ExternalInput")
    dtbd = nc.dram_tensor("dtb", [128, R], F32R, kind="ExternalInput")
    wj1d = nc.dram_tensor("wj1", [DX, HID], F32R, kind="ExternalInput")
    wj2d = nc.dram_tensor("wj2", [128, NCH * HID], F32R, kind="ExternalInput")
    wfhd = nc.dram_tensor("wfh", [128, NCH * HID], F32R, kind="ExternalInput")
    wf4d = nc.dram_tensor("wf4", [M4P, S * HID], F32R, kind="ExternalInput")
    wf2d = nc.dram_tensor("wf2", [128, NCH * HID], F32R, kind="ExternalInput")
    wo1d = nc.dram_tensor("wo1", [128, NCH * HID], F32R, kind="ExternalInput")
    wodtd = nc.dram_tensor("wodt", [1, HID], F32R, kind="ExternalInput")
    wo2d = nc.dram_tensor("wo2", [128, NCH * DY], F32R, kind="ExternalInput")
    biasd = nc.dram_tensor("bias", [128, 16], F32, kind="ExternalInput")
    yjd = nc.dram_tensor("yj", [DY, R], F32, kind="ExternalOutput")
    ybd = nc.dram_tensor("yb", [DY, R], F32, kind="ExternalOutput")

    with tile.TileContext(nc) as tc:
        with (
            tc.tile_pool(name="const", bufs=1) as cp,
            tc.tile_pool(name="work", bufs=3) as wp,
            tc.tile_pool(name="psum", bufs=4, space="PSUM") as pp,
        ):
            wj1 = cp.tile([DX, HID], F32R, name="wj1s")
            wj2 = cp.tile([128, NCH * HID], F32R, name="wj2s")
            wfh = cp.tile([128, NCH * HID], F32R, name="wfhs")
            wf4 = cp.tile([M4P, S * HID], F32R, name="wf4s")
            wf2 = cp.tile([128, NCH * HID], F32R, name="wf2s")
            wo1 = cp.tile([128, NCH * HID], F32R, name="wo1s")
            wodt = cp.tile([1, HID], F32R, name="wodts")
            wo2 = cp.tile([128, NCH * DY], F32R, name="wo2s")
            bias = cp.tile([128, 16], F32, name="biass")
            for sb, dr in ((wj1, wj1d), (wj2, wj2d), (wfh, wfhd),
                           (wf4, wf4d), (wf2, wf2d), (wo1, wo1d),
                           (wodt, wodtd), (wo2, wo2d), (bias, biasd)):
                nc.sync.dma_start(sb[:], dr[:])

            JB1, JB2, FB1, OB1 = 0, 4, 8, 12  # bias column groups

            def body():
                for pair in range(NRT // 2):
                    rts = (2 * pair, 2 * pair + 1)
                    m4 = {}
                    dtb = {}
                    g = {}
                    t = {}
                    td = {}
                    for rt in rts:
                        m4[rt] = wp.tile([M4P, RT], F32R, name="m4t",
                                         tag="m4t")
                        dtb[rt] = wp.tile([128, RT], F32R, name="dtbt",
                                          tag="dtbt")
                        g[rt] = wp.tile([128, NCH * RT], F32R, name="gt",
                                        tag="gt")
                        t[rt] = wp.tile([128, NCH * RT], F32R, name="tt",
                                        tag="tt")
                        td[rt] = wp.tile([128, NCH * RT], F32R, name="tdt",
                                         tag="tdt")
                        nc.sync.dma_start(m4[rt][:],
                                          m4d[:, rt * RT:(rt + 1) * RT])
                        nc.sync.dma_start(dtb[rt][:],
                                          dtbd[:, rt * RT:(rt + 1) * RT])

                    def lay(dst, w, rhs_chunks, bias_col, extra=None):
                        """tanh layer: dst chunks = tanh(W^T @ rhs + bias).

                        w: (128, NCH*HID) chunked weights; rhs_chunks:
                        list of NCH (128, RT) APs; extra: (lhsT, rhs)
                        appended to the contraction. Returns psum tiles.
                        """
                        for c in range(NCH):
                            ps = pp.tile([128, RT], F32, name="p1",
                                         tag="p1")
                            nk = len(rhs_chunks)
                            last = nk - 1 if extra is None else nk
                            for k in range(nk):
                                nc.tensor.matmul(
                                    ps[:],
                                    w[:, k * HID + c * 128:
                                      k * HID + (c + 1) * 128],
                                    rhs_chunks[k],
                                    start=(k == 0), stop=(k == last))
                            if extra is not None:
                                elhs, erhs = extra
                                nc.tensor.matmul(
                                    ps[:],
                                    elhs[:, c * 128:(c + 1) * 128],
                                    erhs,
                                    start=False, stop=True)
                            nc.scalar.activation(
                                dst[:, c * RT:(c + 1) * RT], ps[:], TANH,
                                bias=bias[:, bias_col + c:bias_col + c + 1])

                    def out_phase1(rt, aug):
                        """OL1: t = tanh(oW1^T G + ob1)."""
                        gch = [g[rt][:, k * RT:(k + 1) * RT]
                               for k in range(NCH)]
                        extra = None
                        if aug and use_aug:
                            extra = (wodt, dtb[rt][0:1, :])
                        lay(t[rt], wo1, gch, OB1, extra=extra)

                    def out_phase2(rt, dst_dram):
                        """OL2 + copy + DMA."""
                        ps = pp.tile([DY, RT], F32, name="p2", tag="p2")
                        for k in range(NCH):
                            nc.tensor.matmul(
                                ps[:],
                                wo2[:, k * DY:(k + 1) * DY],
                                t[rt][:, k * RT:(k + 1) * RT],
                                start=(k == 0), stop=(k == NCH - 1))
                        yt = wp.tile([DY, RT], F32, name="yt", tag="yt")
                        nc.vector.tensor_copy(yt[:], ps[:])
                        nc.sync.dma_start(
                            dst_dram[:, rt * RT:(rt + 1) * RT], yt[:])

                    # jump network + first prediction (A/B phase-split
                    # so PE never waits on a tanh tail)
                    for rt in rts:
                        # JL1: K=DX single chunk
                        for c in range(NCH):
                            ps = pp.tile([128, RT], F32, name="p1", tag="p1")
                            nc.tensor.matmul(
                                ps[:], wj1[:, c * 128:(c + 1) * 128],
                                m4[rt][0:DX, :], start=True, stop=True)
                            nc.scalar.activation(
                                t[rt][:, c * RT:(c + 1) * RT], ps[:], TANH,
                                bias=bias[:, JB1 + c:JB1 + c + 1])
                    for rt in rts:
                        # JL2 -> G = H0
                        tch = [t[rt][:, k * RT:(k + 1) * RT]
                               for k in range(NCH)]
                        for c in range(NCH):
                            ps = pp.tile([128, RT], F32, name="p2", tag="p2")
                            for k in range(NCH):
                                nc.tensor.matmul(
                                    ps[:],
                                    wj2[:, k * HID + c * 128:
                                        k * HID + (c + 1) * 128],
                                    tch[k], start=(k == 0),
                                    stop=(k == NCH - 1))
                            nc.scalar.activation(
                                g[rt][:, c * RT:(c + 1) * RT], ps[:], TANH,
                                bias=bias[:, JB2 + c:JB2 + c + 1])
                    for rt in rts:
                        out_phase1(rt, aug=False)
                    for rt in rts:
                        out_phase2(rt, yjd)

                    # Euler substeps.  Emission order is chosen so the
                    # PE stream per substep is [L1_A, L1_B, L2_A, L2_B]:
                    # while A's tanh/mul tail runs on ACT/DVE, the PE is
                    # busy with B's L1, and vice versa — no PE bubbles.
                    for s in range(S):
                        for rt in rts:
                            gch = [g[rt][:, k * RT:(k + 1) * RT]
                                   for k in range(NCH)]
                            lay(t[rt], wfh, gch, FB1,
                                extra=(wf4[:, s * HID:(s + 1) * HID],
                                       m4[rt][:]))
                            for c in range(NCH):
                                nc.vector.tensor_mul(
                                    td[rt][:, c * RT:(c + 1) * RT],
                                    t[rt][:, c * RT:(c + 1) * RT],
                                    dtb[rt][:])
                        for rt in rts:
                            tdch = [td[rt][:, k * RT:(k + 1) * RT]
                                    for k in range(NCH)]
                            for c in range(NCH):
                                ps = pp.tile([128, RT], F32, name="p2",
                                             tag="p2")
                                for k in range(NCH):
                                    nc.tensor.matmul(
                                        ps[:],
                                        wf2[:, k * HID + c * 128:
                                            k * HID + (c + 1) * 128],
                                        tdch[k], start=(k == 0),
                                        stop=(k == NCH - 1))
                                nc.vector.tensor_add(
                                    g[rt][:, c * RT:(c + 1) * RT],
                                    g[rt][:, c * RT:(c + 1) * RT],
                                    ps[:])

                    # prediction just before the next jump
                    for rt in rts:
                        out_phase1(rt, aug=True)
                    for rt in rts:
                        out_phase2(rt, ybd)

            if loop_n is None:
                body()
            else:
                with tc.For_i(0, loop_n, 1):
                    body()

    nc.compile()
    return nc


def _chunk(w):
    """(K, N) -> (128, (K/128)*N), K-chunk kc at columns [kc*N:(kc+1)*N]."""
    k, n = w.shape
    return np.ascontiguousarray(
        w.reshape(k // 128, 128, n).transpose(1, 0, 2).reshape(128, -1))


def _prepare(times, values, jW1, jb1, jW2, jb2, fW1, fb1, fW2, fb2,
             oW1, ob1, oW2, ob2, S):
    f32 = np.float32
    t_next = np.concatenate([times[:, 1:], times[:, -1:]], axis=1)
    dt = ((t_next - times) / f32(S)).astype(f32)

    w64 = {k: np.asarray(v, np.float64) for k, v in dict(
        jW1=jW1, jW2=jW2, fW1=fW1, fW2=fW2, oW1=oW1, oW2=oW2,
        jb1=jb1, jb2=jb2, fb1=fb1, fb2=fb2, ob1=ob1).items()}
    v = w64["fW1"][:HID].T @ w64["fb2"]          # (HID,)

    wf4 = np.empty((M4P, S * HID), f32)
    for s in range(S):
        blk = np.empty((M4P, HID), np.float64)
        blk[0:DX] = w64["fW1"][HID:HID + DX]
        blk[DX] = w64["fW1"][HID + DX]                       # t_i row
        blk[DX + 1] = (w64["fW1"][HID + DX + 1]
                       + s * (w64["fW1"][HID + DX] + v))     # dt row
        wf4[:, s * HID:(s + 1) * HID] = blk.astype(f32)

    wodt = (f32(S) * (w64["oW1"].T @ w64["fb2"])).astype(f32)[None, :]

    bias = np.zeros((128, 16), f32)
    for col, b in ((0, jb1), (4, jb2), (8, fb1), (12, ob1)):
        bias[:, col:col + 4] = np.asarray(b, f32).reshape(NCH, 128).T

    shared = {
        "wj1": np.ascontiguousarray(np.asarray(jW1, f32)),
        "wj2": _chunk(np.asarray(jW2, f32)),
        "wfh": _chunk(np.asarray(fW1, f32)[:HID]),
        "wf4": wf4,
        "wf2": _chunk(np.asarray(fW2, f32)),
        "wo1": _chunk(np.asarray(oW1, f32)),
        "wodt": wodt,
        "wo2": _chunk(np.asarray(oW2, f32)),
        "bias": bias,
    }

    in_maps = []
    for c in range(NCORES):
        sl = slice(c * BLOC, (c + 1) * BLOC)
        m4 = np.empty((M4P, R), f32)
        m4[0:DX] = values[sl].reshape(R, DX).T
        m4[DX] = times[sl].reshape(R)
        m4[DX + 1] = dt[sl].reshape(R)
        dtb = np.ascontiguousarray(
            np.broadcast_to(dt[sl].reshape(R), (128, R)))
        in_maps.append({"m4": m4, "dtb": dtb, **shared})
    return in_maps


def _assemble(results, ob2):
    f32 = np.float32
    ob2 = np.asarray(ob2, f32)

    def gather(name):
        arr = np.stack([results[c][name] for c in range(NCORES)])
        return (arr.transpose(0, 2, 1)              # (8, R, DY)
                .reshape(B, NOBS, DY).astype(f32))

    preds = gather("yj") + ob2
    yb = gather("yb") + ob2
    pb = np.zeros_like(preds)
    pb[:, 1:] = yb[:, :-1]
    return preds, pb


def run_on_hw(inputs, loop_n=None, **run_kwargs):
    """Compile (cached) + run on all 8 cores; returns BassKernelResults.
    loop_n wraps the body in an on-device repeat loop (for timing)."""
    times = np.asarray(inputs["times"], np.float32)
    values = np.asarray(inputs["values"], np.float32)
    S = int(inputs["n_steps"])
    use_aug = bool(np.any(np.asarray(inputs["fb2"])))
    key = (S, loop_n, use_aug)
    if key not in _prog_cache:
        _prog_cache[key] = _build(S, loop_n=loop_n, use_aug=use_aug)
    nc = _prog_cache[key]
    in_maps = _prepare(
        times, values, inputs["jW1"], inputs["jb1"], inputs["jW2"],
        inputs["jb2"], inputs["fW1"], inputs["fb1"], inputs["fW2"],
        inputs["fb2"], inputs["oW1"], inputs["ob1"], inputs["oW2"],
        inputs["ob2"], S)
    res = run_bass_kernel_spmd(nc, in_maps, core_ids=list(range(NCORES)),
                               **run_kwargs)
    return res


def kernel(**inputs):
    res = run_on_hw(inputs)
    return _assemble(res.results, inputs["ob2"])
